# revision 5
# baseline (speedup 1.0000x reference)
"""MoE-LoRA Linear kernel for 8x Trainium2 NeuronCores.

Math: out = x @ W^T + bias + sum_e gate[e] * (x @ A_e^T) @ B_e^T
  x [4,2048,4096], W [4096,4096], A [8,8,4096], B [8,4096,8].
  gate = softmax(router(expert_embed)) top-2 masked * scaling (per-task
  scalars: 8 numbers).

The gate is a per-task constant, so the whole LoRA term is a rank-64
update to W: the host folds W' = W + B @ diag(gate) @ A (one small
sgemm) and the device runs a pure GEMM: out = x @ W'^T + bias.

Device strategy (data-parallel over the 8192 tokens, 1024/core):
  - host pre-transposes and casts to bf16: xT [4096,1024] per core,
    W'T [4096,4096] replicated. bf16 halves HBM traffic; PSUM
    accumulates fp32. Output is stored bf16 and widened on host
    (adds ~0.1% rel err; gate is 2e-2).
  - per core: x^T resident in SBUF (8.4 MB); W' streamed once as
    [128,1024] o-pair tiles retained across both token halves.
  - startup: the first x chunk and small W chunks ride the sync queue
    (its HWDGE ring starts descriptors ~0.7us earlier than scalar's),
    and the first chain begins as two N=256 matmuls so the first real
    matmul only needs a [128,256] W chunk + [128,128] x chunk.
  - PE warm-up: full-duty N=64 matmuls on a zeroed scratch tile keep
    the PE busy from the preamble until real data lands, so the HAM
    clock gate (4096-cycle activity window) lifts 1.2->2.4 GHz as
    early as possible.
  - tail: the final token-tile's two chains run serially (j0 fully
    before j1) so j0's eviction+store hide under j1's matmuls; j1's
    last k-step is split into two N=256 matmuls so the final eviction
    and store cover only [128,256].
"""

import numpy as np

B_, S, D = 4, 2048, 4096
O = 4096
N_CORES = 8
TOKENS = B_ * S
T = TOKENS // N_CORES  # tokens per core
NUM_EXPERTS = 8
TOP_K = 2
SCALING = 16.0 / 64.0
R = 64  # total LoRA rank (8 experts x 8)

_BUILT = None


def _build():
    import concourse.bacc as bacc
    import concourse.mybir as mybir
    from concourse.bass import ts
    from concourse.tile import TileContext

    dt = mybir.dt
    f32 = dt.float32
    bf16 = dt.bfloat16
    P = 128
    DT = D // P          # 32 d-tiles
    OTILE = 512
    NOP = O // (2 * OTILE)   # 4 o-pairs
    TH = 2                   # token halves
    TQ = 4                   # token tiles per half
    WARMUP = 52

    nc = bacc.Bacc("TRN2", target_bir_lowering=False, debug=False)
    xT = nc.dram_tensor("xT", [D, T], bf16, kind="ExternalInput")
    wT = nc.dram_tensor("WT", [D, O], bf16, kind="ExternalInput")
    bias_d = nc.dram_tensor("BIAS", [1, O], f32, kind="ExternalInput")
    out = nc.dram_tensor("OUT", [T, O], bf16, kind="ExternalOutput")

    with TileContext(nc) as tc:
        with (
            nc.allow_low_precision("final output is stored bf16 by design"),
            tc.tile_pool(name="resident", bufs=1) as res,
            tc.tile_pool(name="wpool", bufs=44) as wpool,
            tc.tile_pool(name="opool", bufs=10) as opool,
        ):
            x_sb = res.tile([P, DT, T], bf16, tag="x_sb")
            bias_sb = res.tile([P, O], f32, tag="bias_sb")
            scratch = res.tile([P, 64], bf16, tag="scratch")

            def store(o_t, tok, osl, eng):
                eng.dma_start(out[ts(tok, P), osl], o_t[:])

            with tc.tile_pool(name="psum", bufs=8, space="PSUM") as pp:
                # First compute group's PSUMs, hoisted so the warm-up can
                # target one of them before any data arrives.
                psums0 = [
                    [
                        pp.tile(
                            [P, OTILE], f32, tag="pout",
                            name=f"pout_0_0_{t}_{j}",
                        )
                        for j in range(2)
                    ]
                    for t in range(TQ)
                ]
                # PE warm-up: back-to-back N=64 matmuls on the zeroed
                # scratch tile (full PE duty, unlike tiny-N warmups which
                # leave the array mostly idle and can miss the HAM
                # activity window). Spans the DMA fill so the first real
                # matmuls run at 2.4 GHz. Results land in a PSUM corner
                # that the first real start=True matmul overwrites.
                nc.vector.memzero(scratch[:])
                for _ in range(WARMUP):
                    nc.tensor.matmul(
                        psums0[0][0][0:64, 0:64], lhsT=scratch[:],
                        rhs=scratch[:], start=True, stop=True,
                    )
                for opi in range(NOP):
                    first = opi == 0
                    last = opi == NOP - 1
                    w_tiles = [
                        wpool.tile(
                            [P, 2 * OTILE], bf16, tag="w_t",
                            name=f"w_{opi}_{d}",
                        )
                        for d in range(DT)
                    ]
                    for th in range(TH):
                        if first and th == 0:
                            psums = psums0
                        else:
                            psums = [
                                [
                                    pp.tile(
                                        [P, OTILE], f32, tag="pout",
                                        name=f"pout_{opi}_{th}_{t}_{j}",
                                    )
                                    for j in range(2)
                                ]
                                for t in range(TQ)
                            ]
                        if last and th == 1:
                            # ---- final group: restructured for a short
                            # kernel tail (see module docstring) ----
                            _final_group(
                                nc, ts, tc, psums, w_tiles, x_sb, bias_sb,
                                opool, opi, th, DT, OTILE, P, store,
                            )
                            continue
                        tgroups = [tuple(range(TQ))]
                        for tg in tgroups:
                            for dti in range(DT):
                                dsl = slice(dti * P, (dti + 1) * P)
                                opsl = slice(
                                    opi * 2 * OTILE, (opi + 1) * 2 * OTILE
                                )
                                if th == 0 and tg[0] == 0:
                                    # W stream rides the sync queue; the
                                    # very first tile is chunked small so
                                    # the first matmuls start early
                                    if first and dti == 0:
                                        nc.sync.dma_start(
                                            w_tiles[0][:, 0:256],
                                            wT[0:P, 0:256],
                                        )
                                        nc.sync.dma_start(
                                            x_sb[:, 0, 0:P], xT[0:P, 0:P]
                                        )
                                        nc.sync.dma_start(
                                            w_tiles[0][:, 256:512],
                                            wT[0:P, 256:512],
                                        )
                                        nc.sync.dma_start(
                                            x_sb[:, 0, P:T // 2],
                                            xT[0:P, P:T // 2],
                                        )
                                        nc.sync.dma_start(
                                            w_tiles[0][:, 512:1024],
                                            wT[0:P, 512:1024],
                                        )
                                    else:
                                        nc.sync.dma_start(
                                            w_tiles[dti][:], wT[dsl, opsl]
                                        )
                                if first and th == 0:
                                    # x first-half d1..31 on the scalar
                                    # queue (d0 already on sync above)
                                    if dti > 0:
                                        nc.scalar.dma_start(
                                            x_sb[:, dti, 0:T // 2],
                                            xT[dsl, 0:T // 2],
                                        )
                                for t in tg:
                                    tok = th * TQ + t
                                    if (
                                        first and th == 0 and dti == 0
                                        and t == 0
                                    ):
                                        # first chain starts as two N=256
                                        # matmuls: [0:256] opens the bank
                                        # (start=True clears has_written
                                        # for the whole bank), [256:512]
                                        # follows with start=False and
                                        # overwrites its fresh half
                                        nc.tensor.matmul(
                                            psums[0][0][:, 0:256],
                                            lhsT=x_sb[:, 0, 0:P],
                                            rhs=w_tiles[0][:, 0:256],
                                            start=True, stop=False,
                                        )
                                        nc.tensor.matmul(
                                            psums[0][0][:, 256:512],
                                            lhsT=x_sb[:, 0, 0:P],
                                            rhs=w_tiles[0][:, 256:512],
                                            start=False, stop=False,
                                        )
                                        nc.tensor.matmul(
                                            psums[0][1][:],
                                            lhsT=x_sb[:, 0, 0:P],
                                            rhs=w_tiles[0][:, ts(1, OTILE)],
                                            start=True, stop=False,
                                        )
                                        continue
                                    for j in range(2):
                                        nc.tensor.matmul(
                                            psums[t][j][:],
                                            lhsT=x_sb[:, dti, ts(tok, P)],
                                            rhs=w_tiles[dti][
                                                :, ts(j, OTILE)
                                            ],
                                            start=(dti == 0),
                                            stop=(dti == DT - 1),
                                        )
                            if first and th == 0:
                                # bias chunks: tail of the scalar stream,
                                # done long before the first eviction
                                for c in range(O // OTILE):
                                    bsl = slice(c * OTILE, (c + 1) * OTILE)
                                    nc.scalar.dma_start(
                                        bias_sb[:, bsl],
                                        bias_d[:, bsl].to_broadcast(
                                            (P, OTILE)
                                        ),
                                    )
                                # x second-half d-tiles 0..3 (needed right
                                # at tokhalf-1 start): tail of the sync
                                # stream, arriving just ahead of use
                                for k in range(4):
                                    ksl = slice(k * P, (k + 1) * P)
                                    nc.sync.dma_start(
                                        x_sb[:, k, T // 2:T],
                                        xT[ksl, T // 2:T],
                                    )
                            for ei, (t, j) in enumerate(
                                (t, j) for t in tg for j in range(2)
                            ):
                                tok = th * TQ + t
                                osl = slice(
                                    (2 * opi + j) * OTILE,
                                    (2 * opi + j + 1) * OTILE,
                                )
                                o_t = opool.tile(
                                    [P, OTILE], bf16, tag="o_t"
                                )
                                nc.vector.tensor_add(
                                    out=o_t[:], in0=psums[t][j][:],
                                    in1=bias_sb[:, osl],
                                )
                                store(o_t, tok, osl, nc.scalar)
                                if first and th == 0 and ei > 0:
                                    # x second-half d-tiles 4..31: behind
                                    # the (sem-gated) stores so their
                                    # transfers land after tokhalf 0's
                                    # window, ahead of their consumers
                                    for k in range(4 * ei, 4 * ei + 4):
                                        if k < DT:
                                            ksl = slice(k * P, (k + 1) * P)
                                            nc.scalar.dma_start(
                                                x_sb[:, k, T // 2:T],
                                                xT[ksl, T // 2:T],
                                            )

    nc.compile()
    return nc


def _final_group(
    nc, ts, tc, psums, w_tiles, x_sb, bias_sb, opool, opi, th,
    DT, OTILE, P, store,
):
    """Last o-pair, second token half: ordered so the kernel tail after
    the very last matmul is one [128,256] eviction + one small store.

    - tokens 0..2: interleaved j0/j1 chains as usual (x-tile shared).
    - token 3 j0: full chain alone, evicted+stored while j1 runs.
    - token 3 j1: full chain, but the last k-step is two N=256 matmuls
      so the [512:768] half can be evicted/stored one matmul earlier
      than the [768:1024] half.
    """
    import concourse.mybir as mybir

    bf16 = mybir.dt.bfloat16
    TQ = 4

    def evict(t, j, osl, psl=slice(0, OTILE), eng=None):
        tok = th * TQ + t
        o_t = opool.tile(
            [P, psl.stop - psl.start], bf16, tag="o_t",
            name=f"o_f_{t}_{j}_{psl.start}",
        )
        full = slice(
            (2 * opi + j) * OTILE + psl.start,
            (2 * opi + j) * OTILE + psl.stop,
        )
        nc.vector.tensor_add(
            out=o_t[:], in0=psums[t][j][:, psl], in1=bias_sb[:, full],
        )
        store(o_t, tok, full, eng or nc.scalar)

    # tokens 0,1,2: standard interleaved chains
    for dti in range(DT):
        for t in range(3):
            tok = th * TQ + t
            for j in range(2):
                nc.tensor.matmul(
                    psums[t][j][:],
                    lhsT=x_sb[:, dti, ts(tok, P)],
                    rhs=w_tiles[dti][:, ts(j, OTILE)],
                    start=(dti == 0),
                    stop=(dti == DT - 1),
                )
    for t in range(3):
        for j in range(2):
            evict(t, j, None)
    # token 3, j0: complete chain; its eviction+store overlap j1's chain
    tok = th * TQ + 3
    for dti in range(DT):
        nc.tensor.matmul(
            psums[3][0][:],
            lhsT=x_sb[:, dti, ts(tok, P)],
            rhs=w_tiles[dti][:, ts(0, OTILE)],
            start=(dti == 0),
            stop=(dti == DT - 1),
        )
    evict(3, 0, None)
    # token 3, j1: last k-step split into two N=256 matmuls
    for dti in range(DT - 1):
        nc.tensor.matmul(
            psums[3][1][:],
            lhsT=x_sb[:, dti, ts(tok, P)],
            rhs=w_tiles[dti][:, ts(1, OTILE)],
            start=(dti == 0),
            stop=False,
        )
    lhsT_last = x_sb[:, DT - 1, ts(tok, P)]
    nc.tensor.matmul(
        psums[3][1][:, 0:256],
        lhsT=lhsT_last,
        rhs=w_tiles[DT - 1][:, OTILE:OTILE + 256],
        start=False, stop=True,
    )
    evict(3, 1, None, psl=slice(0, 256), eng=nc.sync)
    nc.tensor.matmul(
        psums[3][1][:, 256:512],
        lhsT=lhsT_last,
        rhs=w_tiles[DT - 1][:, OTILE + 256:2 * OTILE],
        start=False, stop=True,
    )
    evict(3, 1, None, psl=slice(256, 512), eng=nc.scalar)


def _get_nc():
    global _BUILT
    if _BUILT is None:
        _BUILT = _build()
    return _BUILT


def _host_prep(x, W, bias, A, B, expert_embed, router_w):
    x = np.asarray(x, dtype=np.float32)
    W = np.asarray(W, dtype=np.float32)
    bias = np.asarray(bias, dtype=np.float32)
    A = np.asarray(A, dtype=np.float32)
    B = np.asarray(B, dtype=np.float32)
    expert_embed = np.asarray(expert_embed, dtype=np.float32)
    router_w = np.asarray(router_w, dtype=np.float32)

    # Router (per-task, 8 scalars)
    logits = (expert_embed[0] @ router_w.T).astype(np.float32)
    e = np.exp(logits - logits.max())
    probs = (e / e.sum()).astype(np.float32)
    sel = np.argsort(-probs, kind="stable")[:TOP_K]
    gate = np.zeros(NUM_EXPERTS, np.float32)
    gate[sel] = probs[sel] * np.float32(SCALING)

    import ml_dtypes

    # Fold the (per-task constant) gated LoRA into W:
    #   W' = W + sum_e gate_e * B_e @ A_e  -- a rank-64 update.
    Bcat = np.ascontiguousarray(B.transpose(1, 0, 2).reshape(O, R))
    Ascaled = (A * gate[:, None, None]).reshape(R, D)
    Wp = W + Bcat @ Ascaled

    WT = np.ascontiguousarray(Wp.T, dtype=ml_dtypes.bfloat16)
    BIAS = np.ascontiguousarray(bias.reshape(1, O), dtype=np.float32)

    xflat = x.reshape(TOKENS, D)
    in_maps = []
    for c in range(N_CORES):
        xt_shard = np.ascontiguousarray(
            xflat[c * T:(c + 1) * T, :].T, dtype=ml_dtypes.bfloat16
        )
        in_maps.append({"xT": xt_shard, "WT": WT, "BIAS": BIAS})
    return in_maps


def _execute(in_maps, trace=False, **kwargs):
    from concourse.bass_utils import run_bass_kernel_spmd

    nc = _get_nc()
    return run_bass_kernel_spmd(
        nc, in_maps, core_ids=list(range(N_CORES)), trace=trace, **kwargs
    )


def kernel(x, W, bias, A, B, expert_embed, router_w):
    in_maps = _host_prep(x, W, bias, A, B, expert_embed, router_w)
    res = _execute(in_maps, trace=False)
    out = np.concatenate(
        [r["OUT"].astype(np.float32) for r in res.results], axis=0
    )
    return out.reshape(B_, S, O)


# revision 7
# speedup vs baseline: 1.0043x; 1.0043x over previous
"""MoE-LoRA Linear kernel for 8x Trainium2 NeuronCores.

Math: out = x @ W^T + bias + sum_e gate[e] * (x @ A_e^T) @ B_e^T
  x [4,2048,4096], W [4096,4096], A [8,8,4096], B [8,4096,8].
  gate = softmax(router(expert_embed)) top-2 masked * scaling (per-task
  scalars: 8 numbers).

The gate is a per-task constant, so the whole LoRA term is a rank-64
update to W: the host folds W' = W + B @ diag(gate) @ A (one small
sgemm) and the device runs a pure GEMM: out = x @ W'^T + bias.

Device strategy (data-parallel over the 8192 tokens, 1024/core):
  - host pre-transposes and casts to bf16: xT [4096,1024] per core,
    W'T [4096,4096] replicated. bf16 halves HBM traffic; PSUM
    accumulates fp32. Output is stored bf16 and widened on host
    (adds ~0.1% rel err; gate is 2e-2).
  - per core: x^T resident in SBUF (8.4 MB); W' streamed once as
    [128,1024] o-pair tiles retained across both token halves.
  - startup: the first x chunk and small W chunks ride the sync queue
    (its HWDGE ring starts descriptors ~0.7us earlier than scalar's),
    and the first chain begins as two N=256 matmuls so the first real
    matmul only needs a [128,256] W chunk + [128,128] x chunk.
  - PE warm-up: full-duty N=64 matmuls on a zeroed scratch tile keep
    the PE busy from the preamble until real data lands, so the HAM
    clock gate (4096-cycle activity window) lifts 1.2->2.4 GHz as
    early as possible.
  - tail: the final token-tile's two chains run serially (j0 fully
    before j1) so j0's eviction+store hide under j1's matmuls; j1's
    last k-step is split into two N=256 matmuls so the final eviction
    and store cover only [128,256].
"""

import numpy as np

B_, S, D = 4, 2048, 4096
O = 4096
N_CORES = 8
TOKENS = B_ * S
T = TOKENS // N_CORES  # tokens per core
NUM_EXPERTS = 8
TOP_K = 2
SCALING = 16.0 / 64.0
R = 64  # total LoRA rank (8 experts x 8)

_BUILT = None


def _build():
    import concourse.bacc as bacc
    import concourse.mybir as mybir
    from concourse.bass import ts
    from concourse.tile import TileContext

    dt = mybir.dt
    f32 = dt.float32
    bf16 = dt.bfloat16
    P = 128
    DT = D // P          # 32 d-tiles
    OTILE = 512
    NOP = O // (2 * OTILE)   # 4 o-pairs
    TH = 2                   # token halves
    TQ = 4                   # token tiles per half
    WARMUP = 42

    nc = bacc.Bacc("TRN2", target_bir_lowering=False, debug=False)
    xT = nc.dram_tensor("xT", [D, T], bf16, kind="ExternalInput")
    wT = nc.dram_tensor("WT", [D, O], bf16, kind="ExternalInput")
    bias_d = nc.dram_tensor("BIAS", [1, O], f32, kind="ExternalInput")
    out = nc.dram_tensor("OUT", [T, O], f32, kind="ExternalOutput")

    with TileContext(nc) as tc:
        with (
            nc.allow_low_precision("final output is stored bf16 by design"),
            tc.tile_pool(name="resident", bufs=1) as res,
            tc.tile_pool(name="wpool", bufs=44) as wpool,
            tc.tile_pool(name="opool", bufs=10) as opool,
        ):
            x_sb = res.tile([P, DT, T], bf16, tag="x_sb")
            bias_sb = res.tile([P, O], f32, tag="bias_sb")
            scratch = res.tile([P, 64], bf16, tag="scratch")

            def store(o_t, tok, osl, eng):
                eng.dma_start(out[ts(tok, P), osl], o_t[:])

            with tc.tile_pool(name="psum", bufs=8, space="PSUM") as pp:
                # First compute group's PSUMs, hoisted so the warm-up can
                # target one of them before any data arrives.
                psums0 = [
                    [
                        pp.tile(
                            [P, OTILE], f32, tag="pout",
                            name=f"pout_0_0_{t}_{j}",
                        )
                        for j in range(2)
                    ]
                    for t in range(TQ)
                ]
                # PE warm-up: back-to-back N=64 matmuls on the zeroed
                # scratch tile (full PE duty, unlike tiny-N warmups which
                # leave the array mostly idle and can miss the HAM
                # activity window). Spans the DMA fill so the first real
                # matmuls run at 2.4 GHz. Results land in a PSUM corner
                # that the first real start=True matmul overwrites.
                nc.vector.memzero(scratch[:])
                for _ in range(WARMUP):
                    nc.tensor.matmul(
                        psums0[0][0][0:64, 0:64], lhsT=scratch[:],
                        rhs=scratch[:], start=True, stop=True,
                    )
                for opi in range(NOP):
                    first = opi == 0
                    last = opi == NOP - 1
                    w_tiles = [
                        wpool.tile(
                            [P, 2 * OTILE], bf16, tag="w_t",
                            name=f"w_{opi}_{d}",
                        )
                        for d in range(DT)
                    ]
                    for th in range(TH):
                        if first and th == 0:
                            psums = psums0
                        else:
                            psums = [
                                [
                                    pp.tile(
                                        [P, OTILE], f32, tag="pout",
                                        name=f"pout_{opi}_{th}_{t}_{j}",
                                    )
                                    for j in range(2)
                                ]
                                for t in range(TQ)
                            ]
                        if last and th == 1:
                            # ---- final group: restructured for a short
                            # kernel tail (see module docstring) ----
                            _final_group(
                                nc, ts, tc, psums, w_tiles, x_sb, bias_sb,
                                opool, opi, th, DT, OTILE, P, store,
                            )
                            continue
                        tgroups = [tuple(range(TQ))]
                        for tg in tgroups:
                            for dti in range(DT):
                                dsl = slice(dti * P, (dti + 1) * P)
                                opsl = slice(
                                    opi * 2 * OTILE, (opi + 1) * 2 * OTILE
                                )
                                if th == 0 and tg[0] == 0:
                                    # W stream rides the sync queue; the
                                    # very first tile is chunked small so
                                    # the first matmuls start early
                                    if first and dti == 0:
                                        nc.sync.dma_start(
                                            w_tiles[0][:, 0:256],
                                            wT[0:P, 0:256],
                                        )
                                        nc.sync.dma_start(
                                            w_tiles[0][:, 256:512],
                                            wT[0:P, 256:512],
                                        )
                                        nc.sync.dma_start(
                                            w_tiles[0][:, 512:1024],
                                            wT[0:P, 512:1024],
                                        )
                                    else:
                                        nc.sync.dma_start(
                                            w_tiles[dti][:], wT[dsl, opsl]
                                        )
                                if first and th == 0:
                                    # x first-half on the scalar queue so
                                    # its completions land in parallel
                                    # with the W chunks on sync; d0 is
                                    # chunked for a low-latency start
                                    if dti == 0:
                                        nc.scalar.dma_start(
                                            x_sb[:, 0, 0:P], xT[0:P, 0:P]
                                        )
                                        nc.scalar.dma_start(
                                            x_sb[:, 0, P:T // 2],
                                            xT[0:P, P:T // 2],
                                        )
                                    else:
                                        nc.scalar.dma_start(
                                            x_sb[:, dti, 0:T // 2],
                                            xT[dsl, 0:T // 2],
                                        )
                                for t in tg:
                                    tok = th * TQ + t
                                    if (
                                        first and th == 0 and dti == 0
                                        and t == 0
                                    ):
                                        # first chain starts as two N=256
                                        # matmuls: [0:256] opens the bank
                                        # (start=True clears has_written
                                        # for the whole bank), [256:512]
                                        # follows with start=False and
                                        # overwrites its fresh half
                                        nc.tensor.matmul(
                                            psums[0][0][:, 0:256],
                                            lhsT=x_sb[:, 0, 0:P],
                                            rhs=w_tiles[0][:, 0:256],
                                            start=True, stop=False,
                                        )
                                        nc.tensor.matmul(
                                            psums[0][0][:, 256:512],
                                            lhsT=x_sb[:, 0, 0:P],
                                            rhs=w_tiles[0][:, 256:512],
                                            start=False, stop=False,
                                        )
                                        nc.tensor.matmul(
                                            psums[0][1][:],
                                            lhsT=x_sb[:, 0, 0:P],
                                            rhs=w_tiles[0][:, ts(1, OTILE)],
                                            start=True, stop=False,
                                        )
                                        continue
                                    for j in range(2):
                                        nc.tensor.matmul(
                                            psums[t][j][:],
                                            lhsT=x_sb[:, dti, ts(tok, P)],
                                            rhs=w_tiles[dti][
                                                :, ts(j, OTILE)
                                            ],
                                            start=(dti == 0),
                                            stop=(dti == DT - 1),
                                        )
                            if first and th == 0:
                                # bias chunks: tail of the scalar stream,
                                # done long before the first eviction
                                for c in range(O // OTILE):
                                    bsl = slice(c * OTILE, (c + 1) * OTILE)
                                    nc.scalar.dma_start(
                                        bias_sb[:, bsl],
                                        bias_d[:, bsl].to_broadcast(
                                            (P, OTILE)
                                        ),
                                    )
                                # x second-half d-tiles 0..3 (needed right
                                # at tokhalf-1 start): tail of the sync
                                # stream, arriving just ahead of use
                                for k in range(4):
                                    ksl = slice(k * P, (k + 1) * P)
                                    nc.sync.dma_start(
                                        x_sb[:, k, T // 2:T],
                                        xT[ksl, T // 2:T],
                                    )
                            for ei, (t, j) in enumerate(
                                (t, j) for t in tg for j in range(2)
                            ):
                                tok = th * TQ + t
                                osl = slice(
                                    (2 * opi + j) * OTILE,
                                    (2 * opi + j + 1) * OTILE,
                                )
                                o_t = opool.tile(
                                    [P, OTILE], f32, tag="o_t"
                                )
                                nc.vector.tensor_add(
                                    out=o_t[:], in0=psums[t][j][:],
                                    in1=bias_sb[:, osl],
                                )
                                store(o_t, tok, osl, nc.scalar)
                                if first and th == 0 and ei > 0:
                                    # x second-half d-tiles 4..31: behind
                                    # the (sem-gated) stores so their
                                    # transfers land after tokhalf 0's
                                    # window, ahead of their consumers
                                    for k in range(4 * ei, 4 * ei + 4):
                                        if k < DT:
                                            ksl = slice(k * P, (k + 1) * P)
                                            nc.scalar.dma_start(
                                                x_sb[:, k, T // 2:T],
                                                xT[ksl, T // 2:T],
                                            )

    nc.compile()
    return nc


def _final_group(
    nc, ts, tc, psums, w_tiles, x_sb, bias_sb, opool, opi, th,
    DT, OTILE, P, store,
):
    """Last o-pair, second token half: ordered so the kernel tail after
    the very last matmul is one [128,256] eviction + one small store.

    - tokens 0..2: interleaved j0/j1 chains as usual (x-tile shared).
    - token 3 j0: full chain alone, evicted+stored while j1 runs.
    - token 3 j1: full chain, but the last k-step is two N=256 matmuls
      so the [512:768] half can be evicted/stored one matmul earlier
      than the [768:1024] half.
    """
    import concourse.mybir as mybir

    f32 = mybir.dt.float32
    TQ = 4

    def evict(t, j, osl, psl=slice(0, OTILE), eng=None):
        tok = th * TQ + t
        o_t = opool.tile(
            [P, psl.stop - psl.start], f32, tag="o_t",
            name=f"o_f_{t}_{j}_{psl.start}",
        )
        full = slice(
            (2 * opi + j) * OTILE + psl.start,
            (2 * opi + j) * OTILE + psl.stop,
        )
        nc.vector.tensor_add(
            out=o_t[:], in0=psums[t][j][:, psl], in1=bias_sb[:, full],
        )
        store(o_t, tok, full, eng or nc.scalar)

    # tokens 0,1,2: standard interleaved chains
    for dti in range(DT):
        for t in range(3):
            tok = th * TQ + t
            for j in range(2):
                nc.tensor.matmul(
                    psums[t][j][:],
                    lhsT=x_sb[:, dti, ts(tok, P)],
                    rhs=w_tiles[dti][:, ts(j, OTILE)],
                    start=(dti == 0),
                    stop=(dti == DT - 1),
                )
    for t in range(3):
        for j in range(2):
            evict(t, j, None)
    # token 3, j0: complete chain; its eviction+store overlap j1's chain
    tok = th * TQ + 3
    for dti in range(DT):
        nc.tensor.matmul(
            psums[3][0][:],
            lhsT=x_sb[:, dti, ts(tok, P)],
            rhs=w_tiles[dti][:, ts(0, OTILE)],
            start=(dti == 0),
            stop=(dti == DT - 1),
        )
    evict(3, 0, None)
    # token 3, j1: last k-step split into two N=256 matmuls
    for dti in range(DT - 1):
        nc.tensor.matmul(
            psums[3][1][:],
            lhsT=x_sb[:, dti, ts(tok, P)],
            rhs=w_tiles[dti][:, ts(1, OTILE)],
            start=(dti == 0),
            stop=False,
        )
    lhsT_last = x_sb[:, DT - 1, ts(tok, P)]
    nc.tensor.matmul(
        psums[3][1][:, 0:256],
        lhsT=lhsT_last,
        rhs=w_tiles[DT - 1][:, OTILE:OTILE + 256],
        start=False, stop=True,
    )
    evict(3, 1, None, psl=slice(0, 256), eng=nc.sync)
    nc.tensor.matmul(
        psums[3][1][:, 256:512],
        lhsT=lhsT_last,
        rhs=w_tiles[DT - 1][:, OTILE + 256:2 * OTILE],
        start=False, stop=True,
    )
    evict(3, 1, None, psl=slice(256, 512), eng=nc.scalar)


def _get_nc():
    global _BUILT
    if _BUILT is None:
        _BUILT = _build()
    return _BUILT


def _host_prep(x, W, bias, A, B, expert_embed, router_w):
    x = np.asarray(x, dtype=np.float32)
    W = np.asarray(W, dtype=np.float32)
    bias = np.asarray(bias, dtype=np.float32)
    A = np.asarray(A, dtype=np.float32)
    B = np.asarray(B, dtype=np.float32)
    expert_embed = np.asarray(expert_embed, dtype=np.float32)
    router_w = np.asarray(router_w, dtype=np.float32)

    # Router (per-task, 8 scalars)
    logits = (expert_embed[0] @ router_w.T).astype(np.float32)
    e = np.exp(logits - logits.max())
    probs = (e / e.sum()).astype(np.float32)
    sel = np.argsort(-probs, kind="stable")[:TOP_K]
    gate = np.zeros(NUM_EXPERTS, np.float32)
    gate[sel] = probs[sel] * np.float32(SCALING)

    import ml_dtypes

    # Fold the (per-task constant) gated LoRA into W:
    #   W' = W + sum_e gate_e * B_e @ A_e  -- a rank-64 update.
    Bcat = np.ascontiguousarray(B.transpose(1, 0, 2).reshape(O, R))
    Ascaled = (A * gate[:, None, None]).reshape(R, D)
    Wp = W + Bcat @ Ascaled

    WT = np.ascontiguousarray(Wp.T, dtype=ml_dtypes.bfloat16)
    BIAS = np.ascontiguousarray(bias.reshape(1, O), dtype=np.float32)

    xflat = x.reshape(TOKENS, D)
    in_maps = []
    for c in range(N_CORES):
        xt_shard = np.ascontiguousarray(
            xflat[c * T:(c + 1) * T, :].T, dtype=ml_dtypes.bfloat16
        )
        in_maps.append({"xT": xt_shard, "WT": WT, "BIAS": BIAS})
    return in_maps


def _execute(in_maps, trace=False, **kwargs):
    from concourse.bass_utils import run_bass_kernel_spmd

    nc = _get_nc()
    return run_bass_kernel_spmd(
        nc, in_maps, core_ids=list(range(N_CORES)), trace=trace, **kwargs
    )


def kernel(x, W, bias, A, B, expert_embed, router_w):
    in_maps = _host_prep(x, W, bias, A, B, expert_embed, router_w)
    res = _execute(in_maps, trace=False)
    out = np.concatenate([r["OUT"] for r in res.results], axis=0)
    return out.reshape(B_, S, O).astype(np.float32, copy=False)


# revision 8
# speedup vs baseline: 1.0112x; 1.0069x over previous
"""MoE-LoRA Linear kernel for 8x Trainium2 NeuronCores.

Math: out = x @ W^T + bias + sum_e gate[e] * (x @ A_e^T) @ B_e^T
  x [4,2048,4096], W [4096,4096], A [8,8,4096], B [8,4096,8].
  gate = softmax(router(expert_embed)) top-2 masked * scaling (per-task
  scalars: 8 numbers).

The gate is a per-task constant, so the whole LoRA term is a rank-64
update to W: the host folds W' = W + B @ diag(gate) @ A (one small
sgemm) and the device runs a pure GEMM: out = x @ W'^T + bias.

Device strategy (data-parallel over the 8192 tokens, 1024/core):
  - host pre-transposes and casts to bf16: xT [4096,1024] per core,
    W'T [4096,4096] replicated. bf16 halves HBM traffic; PSUM
    accumulates fp32; output is stored fp32 (a bf16 output store was
    tried: no measurable win, costs 0.6e-3 rel err).
  - per core: x^T resident in SBUF (8.4 MB); W' streamed once as
    [128,1024] o-pair tiles retained across both token halves.
  - startup: W on the sync queue (first tile chunked in two), x d0
    chunked on scalar -- measured hole-free W/x arrival for the first
    compute group; finer first-wave chunking was tried and starves the
    early W train (slower overall).
  - PE warm-up: full-duty N=96 matmuls on a zeroed scratch tile keep
    the PE busy from the preamble until real data lands, so the HAM
    clock gate (4096-cycle activity window) lifts 1.2->2.4 GHz as
    early as possible.
  - tail: the final token-tile's two chains run serially (j0 fully
    before j1) so j0's eviction+store hide under j1's matmuls; j1's
    last k-step is split into two N=256 matmuls so the final eviction
    and store cover only [128,256].
"""

import numpy as np

B_, S, D = 4, 2048, 4096
O = 4096
N_CORES = 8
TOKENS = B_ * S
T = TOKENS // N_CORES  # tokens per core
NUM_EXPERTS = 8
TOP_K = 2
SCALING = 16.0 / 64.0
R = 64  # total LoRA rank (8 experts x 8)

_BUILT = None


def _build():
    import concourse.bacc as bacc
    import concourse.mybir as mybir
    from concourse.bass import ts
    from concourse.tile import TileContext

    dt = mybir.dt
    f32 = dt.float32
    bf16 = dt.bfloat16
    P = 128
    DT = D // P          # 32 d-tiles
    OTILE = 512
    NOP = O // (2 * OTILE)   # 4 o-pairs
    TH = 2                   # token halves
    TQ = 4                   # token tiles per half
    WARMUP = 50

    nc = bacc.Bacc("TRN2", target_bir_lowering=False, debug=False)
    xT = nc.dram_tensor("xT", [D, T], bf16, kind="ExternalInput")
    wT = nc.dram_tensor("WT", [D, O], bf16, kind="ExternalInput")
    bias_d = nc.dram_tensor("BIAS", [1, O], f32, kind="ExternalInput")
    out = nc.dram_tensor("OUT", [T, O], f32, kind="ExternalOutput")

    with TileContext(nc) as tc:
        with (
            nc.allow_low_precision("final output is stored bf16 by design"),
            tc.tile_pool(name="resident", bufs=1) as res,
            tc.tile_pool(name="wpool", bufs=44) as wpool,
            tc.tile_pool(name="opool", bufs=10) as opool,
        ):
            x_sb = res.tile([P, DT, T], bf16, tag="x_sb")
            bias_sb = res.tile([P, O], f32, tag="bias_sb")
            scratch = res.tile([P, 96], bf16, tag="scratch")

            def store(o_t, tok, osl, eng):
                eng.dma_start(out[ts(tok, P), osl], o_t[:])

            with tc.tile_pool(name="psum", bufs=8, space="PSUM") as pp:
                # First compute group's PSUMs, hoisted so the warm-up can
                # target one of them before any data arrives.
                psums0 = [
                    [
                        pp.tile(
                            [P, OTILE], f32, tag="pout",
                            name=f"pout_0_0_{t}_{j}",
                        )
                        for j in range(2)
                    ]
                    for t in range(TQ)
                ]
                # PE warm-up: back-to-back N=64 matmuls on the zeroed
                # scratch tile (full PE duty, unlike tiny-N warmups which
                # leave the array mostly idle and can miss the HAM
                # activity window). Spans the DMA fill so the first real
                # matmuls run at 2.4 GHz. Results land in a PSUM corner
                # that the first real start=True matmul overwrites.
                nc.vector.memzero(scratch[:])
                for _ in range(WARMUP):
                    nc.tensor.matmul(
                        psums0[0][0][0:96, 0:96], lhsT=scratch[:],
                        rhs=scratch[:], start=True, stop=True,
                    )
                for opi in range(NOP):
                    first = opi == 0
                    last = opi == NOP - 1
                    w_tiles = [
                        wpool.tile(
                            [P, 2 * OTILE], bf16, tag="w_t",
                            name=f"w_{opi}_{d}",
                        )
                        for d in range(DT)
                    ]
                    for th in range(TH):
                        if first and th == 0:
                            psums = psums0
                        else:
                            psums = [
                                [
                                    pp.tile(
                                        [P, OTILE], f32, tag="pout",
                                        name=f"pout_{opi}_{th}_{t}_{j}",
                                    )
                                    for j in range(2)
                                ]
                                for t in range(TQ)
                            ]
                        if last and th == 1:
                            # ---- final group: restructured for a short
                            # kernel tail (see module docstring) ----
                            _final_group(
                                nc, ts, tc, psums, w_tiles, x_sb, bias_sb,
                                opool, opi, th, DT, OTILE, P, store,
                            )
                            continue
                        tgroups = [tuple(range(TQ))]
                        for tg in tgroups:
                            for dti in range(DT):
                                dsl = slice(dti * P, (dti + 1) * P)
                                opsl = slice(
                                    opi * 2 * OTILE, (opi + 1) * 2 * OTILE
                                )
                                if th == 0 and tg[0] == 0:
                                    # W stream rides the sync queue; the
                                    # very first tile is chunked small so
                                    # the first matmuls start early
                                    if first and dti == 0:
                                        for c in range(2):
                                            wsl = slice(
                                                opi * 2 * OTILE + c * OTILE,
                                                opi * 2 * OTILE
                                                + (c + 1) * OTILE,
                                            )
                                            nc.sync.dma_start(
                                                w_tiles[0][:, ts(c, OTILE)],
                                                wT[0:P, wsl],
                                            )
                                    else:
                                        nc.sync.dma_start(
                                            w_tiles[dti][:], wT[dsl, opsl]
                                        )
                                if first and th == 0:
                                    # x first-half on the scalar queue so
                                    # its completions land in parallel
                                    # with the W chunks on sync; d0 is
                                    # chunked for a low-latency start
                                    if dti == 0:
                                        for c in range(2):
                                            csl = slice(
                                                c * 256, (c + 1) * 256
                                            )
                                            nc.scalar.dma_start(
                                                x_sb[:, 0, csl],
                                                xT[0:P, csl],
                                            )
                                    else:
                                        nc.scalar.dma_start(
                                            x_sb[:, dti, 0:T // 2],
                                            xT[dsl, 0:T // 2],
                                        )
                                for t in tg:
                                    tok = th * TQ + t
                                    for j in range(2):
                                        nc.tensor.matmul(
                                            psums[t][j][:],
                                            lhsT=x_sb[:, dti, ts(tok, P)],
                                            rhs=w_tiles[dti][
                                                :, ts(j, OTILE)
                                            ],
                                            start=(dti == 0),
                                            stop=(dti == DT - 1),
                                        )
                            if first and th == 0:
                                # bias chunks: tail of the scalar stream,
                                # done long before the first eviction
                                for c in range(O // OTILE):
                                    bsl = slice(c * OTILE, (c + 1) * OTILE)
                                    nc.scalar.dma_start(
                                        bias_sb[:, bsl],
                                        bias_d[:, bsl].to_broadcast(
                                            (P, OTILE)
                                        ),
                                    )
                                # x second-half d-tiles 0..3 (needed right
                                # at tokhalf-1 start): tail of the sync
                                # stream, arriving just ahead of use
                                for k in range(4):
                                    ksl = slice(k * P, (k + 1) * P)
                                    nc.sync.dma_start(
                                        x_sb[:, k, T // 2:T],
                                        xT[ksl, T // 2:T],
                                    )
                            for ei, (t, j) in enumerate(
                                (t, j) for t in tg for j in range(2)
                            ):
                                tok = th * TQ + t
                                osl = slice(
                                    (2 * opi + j) * OTILE,
                                    (2 * opi + j + 1) * OTILE,
                                )
                                o_t = opool.tile(
                                    [P, OTILE], f32, tag="o_t"
                                )
                                nc.vector.tensor_add(
                                    out=o_t[:], in0=psums[t][j][:],
                                    in1=bias_sb[:, osl],
                                )
                                store(o_t, tok, osl, nc.scalar)
                                if first and th == 0 and ei > 0:
                                    # x second-half d-tiles 4..31: behind
                                    # the (sem-gated) stores so their
                                    # transfers land after tokhalf 0's
                                    # window, ahead of their consumers
                                    for k in range(4 * ei, 4 * ei + 4):
                                        if k < DT:
                                            ksl = slice(k * P, (k + 1) * P)
                                            nc.scalar.dma_start(
                                                x_sb[:, k, T // 2:T],
                                                xT[ksl, T // 2:T],
                                            )

    nc.compile()
    return nc


def _final_group(
    nc, ts, tc, psums, w_tiles, x_sb, bias_sb, opool, opi, th,
    DT, OTILE, P, store,
):
    """Last o-pair, second token half: ordered so the kernel tail after
    the very last matmul is one [128,256] eviction + one small store.

    - tokens 0..2: interleaved j0/j1 chains as usual (x-tile shared).
    - token 3 j0: full chain alone, evicted+stored while j1 runs.
    - token 3 j1: full chain, but the last k-step is two N=256 matmuls
      so the [512:768] half can be evicted/stored one matmul earlier
      than the [768:1024] half.
    """
    import concourse.mybir as mybir

    f32 = mybir.dt.float32
    TQ = 4

    def evict(t, j, osl, psl=slice(0, OTILE), eng=None):
        tok = th * TQ + t
        o_t = opool.tile(
            [P, psl.stop - psl.start], f32, tag="o_t",
            name=f"o_f_{t}_{j}_{psl.start}",
        )
        full = slice(
            (2 * opi + j) * OTILE + psl.start,
            (2 * opi + j) * OTILE + psl.stop,
        )
        nc.vector.tensor_add(
            out=o_t[:], in0=psums[t][j][:, psl], in1=bias_sb[:, full],
        )
        store(o_t, tok, full, eng or nc.scalar)

    # tokens 0,1,2: standard interleaved chains
    for dti in range(DT):
        for t in range(3):
            tok = th * TQ + t
            for j in range(2):
                nc.tensor.matmul(
                    psums[t][j][:],
                    lhsT=x_sb[:, dti, ts(tok, P)],
                    rhs=w_tiles[dti][:, ts(j, OTILE)],
                    start=(dti == 0),
                    stop=(dti == DT - 1),
                )
    for t in range(3):
        for j in range(2):
            evict(t, j, None)
    # token 3, j0: complete chain; its eviction+store overlap j1's chain
    tok = th * TQ + 3
    for dti in range(DT):
        nc.tensor.matmul(
            psums[3][0][:],
            lhsT=x_sb[:, dti, ts(tok, P)],
            rhs=w_tiles[dti][:, ts(0, OTILE)],
            start=(dti == 0),
            stop=(dti == DT - 1),
        )
    evict(3, 0, None)
    # token 3, j1: last k-step split into two N=256 matmuls
    for dti in range(DT - 1):
        nc.tensor.matmul(
            psums[3][1][:],
            lhsT=x_sb[:, dti, ts(tok, P)],
            rhs=w_tiles[dti][:, ts(1, OTILE)],
            start=(dti == 0),
            stop=False,
        )
    lhsT_last = x_sb[:, DT - 1, ts(tok, P)]
    nc.tensor.matmul(
        psums[3][1][:, 0:256],
        lhsT=lhsT_last,
        rhs=w_tiles[DT - 1][:, OTILE:OTILE + 256],
        start=False, stop=True,
    )
    evict(3, 1, None, psl=slice(0, 256), eng=nc.sync)
    nc.tensor.matmul(
        psums[3][1][:, 256:512],
        lhsT=lhsT_last,
        rhs=w_tiles[DT - 1][:, OTILE + 256:2 * OTILE],
        start=False, stop=True,
    )
    evict(3, 1, None, psl=slice(256, 512), eng=nc.scalar)


def _get_nc():
    global _BUILT
    if _BUILT is None:
        _BUILT = _build()
    return _BUILT


def _host_prep(x, W, bias, A, B, expert_embed, router_w):
    x = np.asarray(x, dtype=np.float32)
    W = np.asarray(W, dtype=np.float32)
    bias = np.asarray(bias, dtype=np.float32)
    A = np.asarray(A, dtype=np.float32)
    B = np.asarray(B, dtype=np.float32)
    expert_embed = np.asarray(expert_embed, dtype=np.float32)
    router_w = np.asarray(router_w, dtype=np.float32)

    # Router (per-task, 8 scalars)
    logits = (expert_embed[0] @ router_w.T).astype(np.float32)
    e = np.exp(logits - logits.max())
    probs = (e / e.sum()).astype(np.float32)
    sel = np.argsort(-probs, kind="stable")[:TOP_K]
    gate = np.zeros(NUM_EXPERTS, np.float32)
    gate[sel] = probs[sel] * np.float32(SCALING)

    import ml_dtypes

    # Fold the (per-task constant) gated LoRA into W:
    #   W' = W + sum_e gate_e * B_e @ A_e  -- a rank-64 update.
    Bcat = np.ascontiguousarray(B.transpose(1, 0, 2).reshape(O, R))
    Ascaled = (A * gate[:, None, None]).reshape(R, D)
    Wp = W + Bcat @ Ascaled

    WT = np.ascontiguousarray(Wp.T, dtype=ml_dtypes.bfloat16)
    BIAS = np.ascontiguousarray(bias.reshape(1, O), dtype=np.float32)

    xflat = x.reshape(TOKENS, D)
    in_maps = []
    for c in range(N_CORES):
        xt_shard = np.ascontiguousarray(
            xflat[c * T:(c + 1) * T, :].T, dtype=ml_dtypes.bfloat16
        )
        in_maps.append({"xT": xt_shard, "WT": WT, "BIAS": BIAS})
    return in_maps


def _execute(in_maps, trace=False, **kwargs):
    from concourse.bass_utils import run_bass_kernel_spmd

    nc = _get_nc()
    return run_bass_kernel_spmd(
        nc, in_maps, core_ids=list(range(N_CORES)), trace=trace, **kwargs
    )


def kernel(x, W, bias, A, B, expert_embed, router_w):
    in_maps = _host_prep(x, W, bias, A, B, expert_embed, router_w)
    res = _execute(in_maps, trace=False)
    out = np.concatenate([r["OUT"] for r in res.results], axis=0)
    return out.reshape(B_, S, O).astype(np.float32, copy=False)
